# revision 2
# baseline (speedup 1.0000x reference)
"""Block-circulant linear (nn_BlockCirculantLinear) Trainium2 kernel.

Math: out[b,s,n*8+j] = irfft8( sum_i rfft8(x[b,s,i*8:+8]) * rfft8(c[n,i,:]) ) + bias

Strategy (8 NeuronCores, data-parallel over the 8192 batch*seq rows):
  - host: pack x into (core, i=512, k=8, bs=1024) fp16; rfft the weights into
    11 fp16 planes [i, n] with the irfft/8 scaling folded in.
  - device: forward rfft8 of x as ~20 vector ops per (i-chunk, bs-half)
    producing 8 real component planes X[i, bs] (fp16);
    per 128-row bs-tile, 56 fp16 matmuls (K=128 x4 chunks, N=512) accumulate
    the 8 per-frequency output component planes in PSUM (fp32);
    ScalarE copies PSUM -> SBUF fp16; ~20 vector ops do the inverse rfft8 and
    write the 8 phase planes interleaved into the (128, 4096) output tile;
    DMA out. Output gathered and cast to fp32 on host.
"""
import sys

sys.path.insert(0, "/opt/trn_rl_repo")

import numpy as np

B, S, IN_F, OUT_F, BLK = 4, 2048, 4096, 4096, 8
NB_IN, NB_OUT = IN_F // BLK, OUT_F // BLK  # 512, 512
N_CORES = 8
BS_TOT = B * S              # 8192
SH = BS_TOT // N_CORES      # 1024 rows per core
HALF = 512                  # bs processed per forward-FFT sweep
N_CH = NB_IN // 128         # 4 contraction chunks

C_SQ = float(np.sqrt(0.5))

# psum plane -> list of (x component, weight plane) matmul pairs
# x comps: 0:X0r 1:X1r 2:X1i 3:X2r 4:X2i 5:X3r 6:X3i 7:X4r
# w planes: 0:P0 1:P1r 2:P1i 3:P1ni 4:P2r 5:P2i 6:P2ni 7:P3r 8:P3i 9:P3ni 10:P4
PAIRS = [
    [(0, 0)],
    [(1, 1), (2, 3)],
    [(1, 2), (2, 1)],
    [(3, 4), (4, 6)],
    [(3, 5), (4, 4)],
    [(5, 7), (6, 9)],
    [(5, 8), (6, 7)],
    [(7, 10)],
]

_CACHE = {}


def _weight_planes(circ_params):
    cr = np.fft.rfft(circ_params.astype(np.float64), axis=-1)  # (n, i, 5)
    P = [(cr[..., 0].real / 8).T]
    for f in (1, 2, 3):
        P.append((cr[..., f].real / 4).T)
        P.append((cr[..., f].imag / 4).T)
        P.append((-cr[..., f].imag / 4).T)
    P.append((cr[..., 4].real / 8).T)
    # stack into (i_local=128, plane=11, chunk=4, n=512) fp16
    W = np.stack(P, 0).reshape(11, N_CH, 128, NB_OUT)  # (pl, ch, il, n)
    return np.ascontiguousarray(W.transpose(2, 0, 1, 3)).astype(np.float16)


def _bias_comps(bias):
    br = np.fft.rfft(bias.astype(np.float64).reshape(NB_OUT, BLK), axis=-1)  # (n,5)
    comps = [br[:, 0].real / 8]
    for f in (1, 2, 3):
        comps += [br[:, f].real / 4, br[:, f].imag / 4]
    comps.append(br[:, 4].real / 8)
    # order must match psum planes: y0, y1r, y1i, y2r, y2i, y3r, y3i, y4
    return np.stack(comps, 0).astype(np.float16)  # (8, 512)


def _build(with_bias: bool, repeat: int = 1):
    key = (with_bias, repeat)
    if key in _CACHE:
        return _CACHE[key]
    import concourse.bass as bass
    import concourse.mybir as mybir
    import concourse.tile as tile
    from concourse import bacc

    F16 = mybir.dt.float16
    F32 = mybir.dt.float32
    AL = mybir.AluOpType

    nc = bacc.Bacc("TRN2", target_bir_lowering=False, debug=False)
    xt_d = nc.dram_tensor("xt", [NB_IN, BLK, SH], F16, kind="ExternalInput")
    w_d = nc.dram_tensor("w", [128, 11, N_CH, NB_OUT], F16, kind="ExternalInput")
    wb_d = nc.dram_tensor("wb", [8, NB_OUT], F16, kind="ExternalInput")
    out_d = nc.dram_tensor("out", [SH, OUT_F], F16, kind="ExternalOutput")

    with tile.TileContext(nc) as tc:
        with tc.tile_pool(name="wp", bufs=1) as wp, \
             tc.tile_pool(name="xin", bufs=2) as xin, \
             tc.tile_pool(name="xf", bufs=2) as xfp, \
             tc.tile_pool(name="ft", bufs=2) as ft, \
             tc.tile_pool(name="yc", bufs=2) as ycp, \
             tc.tile_pool(name="it", bufs=2) as itp, \
             tc.tile_pool(name="ot", bufs=2) as otp, \
             tc.tile_pool(name="ps", bufs=1, space="PSUM") as ps:

            wt = wp.tile([128, 11, N_CH, NB_OUT], F16)
            nc.sync.dma_start(out=wt, in_=w_d.ap())
            if with_bias:
                wbt = wp.tile([1, 8, NB_OUT], F16, tag="wbt")
                nc.sync.dma_start(out=wbt, in_=wb_d.ap().rearrange("c n -> 1 c n"))
                ones = wp.tile([1, 128], F16, tag="ones")
                nc.vector.memset(ones, 1.0)

            def loop_body():
                for half in range(2):
                    b0 = half * HALF
                    # forward FFT of this bs-half, all 4 i-chunks
                    xf = xfp.tile([128, N_CH, 8, HALF], F16, tag="xf")
                    for ch in range(N_CH):
                        xt = xin.tile([128, BLK, HALF], F16, tag="xt")
                        nc.sync.dma_start(
                            out=xt,
                            in_=xt_d.ap()[ch * 128:(ch + 1) * 128, :, b0:b0 + HALF])
                        x = [xt[:, k, :] for k in range(8)]
                        s, d = [], []
                        for k in range(4):
                            skt = ft.tile([128, HALF], F16, tag=f"s{k}", name=f"s{k}")
                            dkt = ft.tile([128, HALF], F16, tag=f"d{k}", name=f"d{k}")
                            eng = nc.gpsimd if k % 2 == 0 else nc.vector
                            eng.tensor_tensor(skt, x[k], x[k + 4], AL.add)
                            eng2 = nc.gpsimd if k % 2 == 1 else nc.vector
                            eng2.tensor_tensor(dkt, x[k], x[k + 4], AL.subtract)
                            s.append(skt)
                            d.append(dkt)
                        t0 = ft.tile([128, HALF], F16, tag="t0")
                        t1 = ft.tile([128, HALF], F16, tag="t1")
                        u = ft.tile([128, HALF], F16, tag="u")
                        v = ft.tile([128, HALF], F16, tag="v")
                        nc.gpsimd.tensor_tensor(t0, s[0], s[2], AL.add)
                        nc.vector.tensor_tensor(t1, s[1], s[3], AL.add)
                        nc.gpsimd.tensor_tensor(u, d[1], d[3], AL.subtract)
                        nc.vector.tensor_tensor(v, d[1], d[3], AL.add)
                        XF = lambda comp: xf[:, ch, comp, :]
                        nc.vector.tensor_tensor(XF(0), t0, t1, AL.add)
                        nc.gpsimd.tensor_tensor(XF(7), t0, t1, AL.subtract)
                        nc.vector.tensor_tensor(XF(3), s[0], s[2], AL.subtract)
                        nc.gpsimd.tensor_tensor(XF(4), s[3], s[1], AL.subtract)
                        nc.vector.scalar_tensor_tensor(
                            XF(1), u, C_SQ, d[0], AL.mult, AL.add)
                        nc.vector.scalar_tensor_tensor(
                            XF(5), u, -C_SQ, d[0], AL.mult, AL.add)
                        nc.vector.scalar_tensor_tensor(
                            XF(2), v, -C_SQ, d[2], AL.mult, AL.subtract)
                        nc.vector.scalar_tensor_tensor(
                            XF(6), v, -C_SQ, d[2], AL.mult, AL.add)

                    for t in range(HALF // 128):
                        r0 = t * 128
                        psum = [ps.tile([128, NB_OUT], F32, tag=f"ps{c}", name=f"ps{c}")
                                for c in range(8)]
                        for c, pairs in enumerate(PAIRS):
                            if with_bias:
                                nc.tensor.matmul(psum[c], ones[0:1, :],
                                                 wbt[0:1, c, :],
                                                 start=True, stop=False)
                            n_mm = len(pairs) * N_CH
                            k = 0
                            for xc, pl in pairs:
                                for ch in range(N_CH):
                                    nc.tensor.matmul(
                                        psum[c],
                                        xf[:, ch, xc, r0:r0 + 128],
                                        wt[:, pl, ch, :],
                                        start=(k == 0 and not with_bias),
                                        stop=(k == n_mm - 1))
                                    k += 1
                        y = []
                        for c in range(8):
                            yt = ycp.tile([128, NB_OUT], F16, tag=f"y{c}", name=f"y{c}")
                            nc.scalar.copy(yt, psum[c])
                            y.append(yt)
                        T = lambda tag: itp.tile([128, NB_OUT], F16, tag=tag, name=tag)
                        p, q = T("p"), T("q")
                        A0, A1, A2, A3 = T("A0"), T("A1"), T("A2"), T("A3")
                        B0, B2, u2, v2 = T("B0"), T("B2"), T("u2"), T("v2")
                        w1, w2 = T("w1"), T("w2")
                        nc.vector.tensor_tensor(p, y[0], y[7], AL.add)
                        nc.gpsimd.tensor_tensor(q, y[0], y[7], AL.subtract)
                        nc.vector.tensor_tensor(A0, p, y[3], AL.add)
                        nc.gpsimd.tensor_tensor(A2, p, y[3], AL.subtract)
                        nc.vector.tensor_tensor(A1, q, y[4], AL.subtract)
                        nc.gpsimd.tensor_tensor(A3, q, y[4], AL.add)
                        nc.vector.tensor_tensor(u2, y[1], y[5], AL.subtract)
                        nc.gpsimd.tensor_tensor(v2, y[2], y[6], AL.add)
                        nc.vector.tensor_tensor(B0, y[1], y[5], AL.add)
                        nc.gpsimd.tensor_tensor(B2, y[6], y[2], AL.subtract)
                        nc.vector.tensor_tensor(w1, u2, v2, AL.subtract)
                        nc.gpsimd.tensor_tensor(w2, u2, v2, AL.add)
                        ot = otp.tile([128, NB_OUT, 8], F16, tag="ot")
                        nc.vector.tensor_tensor(ot[:, :, 0], A0, B0, AL.add)
                        nc.gpsimd.tensor_tensor(ot[:, :, 4], A0, B0, AL.subtract)
                        nc.vector.tensor_tensor(ot[:, :, 2], A2, B2, AL.add)
                        nc.gpsimd.tensor_tensor(ot[:, :, 6], A2, B2, AL.subtract)
                        nc.vector.scalar_tensor_tensor(
                            ot[:, :, 1], w1, C_SQ, A1, AL.mult, AL.add)
                        nc.vector.scalar_tensor_tensor(
                            ot[:, :, 5], w1, -C_SQ, A1, AL.mult, AL.add)
                        nc.vector.scalar_tensor_tensor(
                            ot[:, :, 3], w2, -C_SQ, A3, AL.mult, AL.add)
                        nc.vector.scalar_tensor_tensor(
                            ot[:, :, 7], w2, C_SQ, A3, AL.mult, AL.add)
                        nc.sync.dma_start(
                            out=out_d.ap()[b0 + r0:b0 + r0 + 128, :],
                            in_=ot.rearrange("p n j -> p (n j)"))

            if repeat > 1:
                with tc.For_i(0, repeat, 1):
                    loop_body()
            else:
                loop_body()

    nc.compile()
    _CACHE[key] = nc
    return nc


def kernel(x, circ_params, bias):
    from concourse.bass_utils import run_bass_kernel_spmd

    x = np.asarray(x)
    w_host = _weight_planes(np.asarray(circ_params))
    bias = np.asarray(bias)
    with_bias = bool(np.any(bias))
    wb_host = _bias_comps(bias)

    # (core, i, k, bs_shard) fp16
    xt_all = np.ascontiguousarray(
        x.reshape(N_CORES, SH, NB_IN, BLK).transpose(0, 2, 3, 1)
    ).astype(np.float16)

    nc = _build(with_bias)
    in_maps = [
        {"xt": xt_all[c], "w": w_host, "wb": wb_host} for c in range(N_CORES)
    ]
    res = run_bass_kernel_spmd(nc, in_maps, list(range(N_CORES)))
    out = np.stack([res.results[c]["out"] for c in range(N_CORES)], 0)
    return out.reshape(B, S, OUT_F).astype(np.float32)
